# revision 7
# baseline (speedup 1.0000x reference)
"""MoE routing kernel for Trainium2 (Bass/Tile), 8 NeuronCores.

DeepSeek-style MoE block: sigmoid router with group-limited top-k (4 groups
of 2 experts, top-2 groups -> all 4 of their experts), 8 routed SwiGLU
experts (H=1024, I=512) with combine weights, plus a shared expert,
N=8192 tokens.

Strategy (v3, "pure-GEMM device"):
  - Group-expert-parallel: each of the 4 router groups is owned by 2 cores;
    the host replicates the reference's fp32 routing, assigns each token's
    rows to its two selected groups' cores (even/odd split), and ALSO
    computes the exact combine weights (sigmoid-score normalization) on the
    host. This is all part of the all-to-all token dispatch that the
    sharding hint sanctions host-side; none of it is device work.
  - The host additionally pre-transposes the token activations, so the
    device kernel is nothing but expert GEMM streaming: no PE transposes,
    no on-chip router, no top-k compare chains. Per core: 2 routed experts
    over RT*128 rows (RT sized exactly from the realized routing, ~2050
    rows) + the shared expert over a dense 1024-token shard.
  - All expert matmuls run in bf16 (weights and activations host-rounded;
    ~1e-3 relative error vs the fp32 reference, well under the 2e-2 gate).
    bf16 keeps the PE at 1 row/cycle even for narrow tails, enables fast
    weight load (FWL), and halves DMA traffic vs fp32.
  - PSUM f32 accumulation throughout; combine weights applied during the
    down-projection drain with per-partition-scalar DVE ops; partial
    outputs summed on the host.
  - x rides the ACT HWDGE DMA ring, weights and stores the SP ring.
"""

import math

import numpy as np
import ml_dtypes

import concourse.bass as bass
import concourse.bacc as bacc
import concourse.tile as tile
from concourse import mybir
from concourse.bass_utils import run_bass_kernel_spmd

F32 = mybir.dt.float32
BF16 = mybir.dt.bfloat16
AF = mybir.ActivationFunctionType
ALU = mybir.AluOpType

B, T, H, I, E = 32, 256, 1024, 512, 8
N = B * T                     # 8192 tokens
NCORES = 8
NTOK = N // NCORES            # 1024 dense tokens per core (shared expert)
HK = H // 128                 # 8 contraction chunks over H
IK = I // 128                 # 4 chunks over I
SCALE = 2.5
BF = ml_dtypes.bfloat16

TRACE = False
LAST_RESULT = None
_NC_CACHE = {}


def _blocks(ntiles):
    """Split ntiles 128-row tiles into blocks of <=4 tiles (<=512 rows)."""
    out = [4] * (ntiles // 4)
    if ntiles % 4:
        out.append(ntiles % 4)
    return out


def _build_kernel(rt):
    """rt: number of 128-row tiles in the routed phase (per core)."""
    R = rt * 128
    nc = bacc.Bacc("TRN2", target_bir_lowering=False)

    xrT_d = nc.dram_tensor("xrT", [H, R], BF16, kind="ExternalInput")
    xsT_d = nc.dram_tensor("xsT", [H, NTOK], BF16, kind="ExternalInput")
    cw_d = nc.dram_tensor("cw", [R, 2], F32, kind="ExternalInput")
    wg_d = nc.dram_tensor("Wg2", [2, H, I], BF16, kind="ExternalInput")
    wu_d = nc.dram_tensor("Wu2", [2, H, I], BF16, kind="ExternalInput")
    wd_d = nc.dram_tensor("Wd2", [2, I, H], BF16, kind="ExternalInput")
    wgs_d = nc.dram_tensor("Wg_s", [H, I], BF16, kind="ExternalInput")
    wus_d = nc.dram_tensor("Wu_s", [H, I], BF16, kind="ExternalInput")
    wds_d = nc.dram_tensor("Wd_s", [I, H], BF16, kind="ExternalInput")
    outr_d = nc.dram_tensor("out_r", [R, H], F32, kind="ExternalOutput")
    outs_d = nc.dram_tensor("out_s", [NTOK, H], F32, kind="ExternalOutput")

    with tile.TileContext(nc) as tc:
        with (
            tc.tile_pool(name="wt", bufs=1) as p_wt,
            tc.tile_pool(name="cw", bufs=1) as p_cw,
            tc.tile_pool(name="xT", bufs=3) as p_xT,
            tc.tile_pool(name="sg", bufs=4) as p_sg,
            tc.tile_pool(name="h", bufs=2) as p_h,
            tc.tile_pool(name="acc", bufs=2) as p_acc,
            tc.tile_pool(name="st", bufs=6) as p_st,
            tc.tile_pool(name="psA", bufs=4, space="PSUM") as p_psA,
            tc.tile_pool(name="psY", bufs=2, space="PSUM") as p_psY,
        ):
            # combine weights: [R, 2] -> [128, rt, 2]
            cw_t = p_cw.tile([128, rt, 2], F32, tag="cw")
            nc.sync.dma_start(
                out=cw_t[:, :, :],
                in_=cw_d.ap().rearrange("(rt p) k -> p rt k", p=128),
            )
            cw_f = cw_t.rearrange("p rt k -> p (rt k)")

            # resident expert weights (bf16); one tag per tile (all live).
            # Chunked DMAs (one per 128-row slab) so the PE can start
            # consuming the first slabs while later ones are in flight.
            def load_gu(dram, idx2, tag):
                t = p_wt.tile([128, HK, I], BF16, tag=tag)
                src = dram.ap() if idx2 is None else dram.ap()[idx2]
                for hk in range(HK):
                    nc.sync.dma_start(
                        out=t[:, hk, :].squeeze(),
                        in_=src[hk * 128:(hk + 1) * 128, :],
                    )
                return t

            def load_wd(dram, idx2, tag):
                t = p_wt.tile([128, IK, H], BF16, tag=tag)
                src = dram.ap() if idx2 is None else dram.ap()[idx2]
                for kc in range(IK):
                    nc.sync.dma_start(
                        out=t[:, kc, :].squeeze(),
                        in_=src[kc * 128:(kc + 1) * 128, :],
                    )
                return t

            wu2 = [load_gu(wu_d, 0, "wu0"), None]
            wg2 = [load_gu(wg_d, 0, "wg0"), None]
            wd2 = [load_wd(wd_d, 0, "wd0"), None]
            wu2[1] = load_gu(wu_d, 1, "wu1")
            wg2[1] = load_gu(wg_d, 1, "wg1")
            wd2[1] = load_wd(wd_d, 1, "wd1")
            wus = load_gu(wus_d, None, "wus")
            wgs = load_gu(wgs_d, None, "wgs")
            wds = load_wd(wds_d, None, "wds")

            def expert_block(xT, ntile, wg, wu, wd, combine):
                """SwiGLU for one expert over one <=512-token block.
                combine(m, y_psum) drains each 128-token down-proj result."""
                TBb = ntile * 128
                h = p_h.tile([128, IK, TBb], BF16, tag="h")
                for ik in range(IK):
                    ps_u = p_psA.tile([128, TBb], F32, tag="gu")
                    for hk in range(HK):
                        nc.tensor.matmul(
                            ps_u[:, :], wu[:, hk, ik * 128:(ik + 1) * 128],
                            xT[:, hk, 0:TBb], start=(hk == 0), stop=(hk == HK - 1),
                        )
                    ps_g = p_psA.tile([128, TBb], F32, tag="gu")
                    for hk in range(HK):
                        nc.tensor.matmul(
                            ps_g[:, :], wg[:, hk, ik * 128:(ik + 1) * 128],
                            xT[:, hk, 0:TBb], start=(hk == 0), stop=(hk == HK - 1),
                        )
                    sg = p_sg.tile([128, TBb], F32, tag="sg")
                    nc.scalar.activation(sg[:, :], ps_g[:, :], AF.Silu)
                    nc.vector.tensor_tensor(
                        h[:, ik, :], sg[:, :], ps_u[:, :], ALU.mult
                    )
                for m in range(ntile):
                    y = p_psY.tile([128, H], F32, tag="y")
                    for ik in range(IK):
                        lhsT = h[:, ik, m * 128:(m + 1) * 128]
                        for nh in range(2):
                            nc.tensor.matmul(
                                y[:, nh * 512:(nh + 1) * 512],
                                lhsT,
                                wd[:, ik, nh * 512:(nh + 1) * 512],
                                start=(ik == 0),
                                stop=(ik == IK - 1),
                            )
                    combine(m, y)

            # ---------------- phase 1: routed rows ----------------
            tt0 = 0
            for ntile in _blocks(rt):
                t0 = tt0 * 128
                TBb = ntile * 128
                xT = p_xT.tile([128, HK, TBb], BF16, tag="xT")
                nc.scalar.dma_start(
                    out=xT[:, :, :],
                    in_=xrT_d.ap().rearrange("(hk p) t -> p hk t", p=128)[
                        :, :, t0:t0 + TBb
                    ],
                )
                acc = p_acc.tile([128, ntile, H], F32, tag="acc")
                for slot in (0, 1):
                    def combine(m, y, slot=slot, acc=acc, tt0=tt0):
                        a = acc[:, m, :].squeeze()
                        col = cw_f[:, (tt0 + m) * 2 + slot:(tt0 + m) * 2 + slot + 1]
                        if slot == 0:
                            nc.vector.tensor_scalar(
                                a, y[:, :], col, None, ALU.mult
                            )
                        else:
                            nc.vector.scalar_tensor_tensor(
                                a, y[:, :], col, a, ALU.mult, ALU.add
                            )
                    expert_block(xT, ntile, wg2[slot], wu2[slot], wd2[slot], combine)
                for m in range(ntile):
                    tt = tt0 + m
                    nc.sync.dma_start(
                        out=outr_d.ap()[tt * 128:(tt + 1) * 128, :],
                        in_=acc[:, m, :].squeeze(),
                    )
                tt0 += ntile

            # ---------------- phase 2: shared expert ----------------
            st0 = 0
            for ntile in _blocks(NTOK // 128):
                t0 = st0 * 128
                TBb = ntile * 128
                xT = p_xT.tile([128, HK, TBb], BF16, tag="xT")
                nc.scalar.dma_start(
                    out=xT[:, :, :],
                    in_=xsT_d.ap().rearrange("(hk p) t -> p hk t", p=128)[
                        :, :, t0:t0 + TBb
                    ],
                )
                def combine(m, y, st0=st0):
                    tt = st0 + m
                    stage = p_st.tile([128, H], F32, tag="st")
                    nc.scalar.activation(stage[:, :], y[:, :], AF.Copy)
                    nc.sync.dma_start(
                        out=outs_d.ap()[tt * 128:(tt + 1) * 128, :],
                        in_=stage[:, :],
                    )
                expert_block(xT, ntile, wgs, wus, wds, combine)
                st0 += ntile

    if not nc.is_finalized():
        nc.finalize()
    return nc


def _get_nc(rt):
    if rt not in _NC_CACHE:
        _NC_CACHE[rt] = _build_kernel(rt)
    return _NC_CACHE[rt]


def kernel(**inputs):
    global LAST_RESULT
    hs = np.asarray(inputs["hidden_states"], np.float32)
    x = np.ascontiguousarray(hs.reshape(N, H))
    gw = np.ascontiguousarray(np.asarray(inputs["gate_w"], np.float32))
    cb = np.ascontiguousarray(np.asarray(inputs["correction_bias"], np.float32))

    # ---- host router: replicate the reference's fp32 group top-2 choice ----
    logits = x @ gw.T                                            # [N, E] f32
    scores = (1.0 / (1.0 + np.exp(-logits.astype(np.float64)))).astype(np.float32)
    sc = scores + cb
    gs = sc.reshape(N, 4, 2).sum(-1, dtype=np.float32)           # [N, 4]
    order = np.argsort(-gs, axis=1, kind="stable")
    sel = np.zeros((N, 4), bool)
    sel[np.arange(N)[:, None], order[:, :2]] = True              # [N, 4] groups
    sel_e = np.repeat(sel, 2, axis=1)                            # [N, E]
    w4 = np.where(sel_e, scores, 0.0).astype(np.float32)
    denom = w4.sum(1, dtype=np.float32) + np.float32(1e-20)
    cw_full = (w4 / denom[:, None] * np.float32(SCALE)).astype(np.float32)

    # ---- shard: rows of group g split even/odd between cores 2g, 2g+1 ----
    core_rows = []
    for c in range(NCORES):
        g, hlf = c // 2, c % 2
        core_rows.append(np.flatnonzero(sel[:, g])[hlf::2])
    rt = max(1, max(int(math.ceil(len(r) / 128)) for r in core_rows))
    R = rt * 128

    xb = x.astype(BF)
    Wg = np.asarray(inputs["Wg"], np.float32).astype(BF)
    Wu = np.asarray(inputs["Wu"], np.float32).astype(BF)
    Wd = np.asarray(inputs["Wd"], np.float32).astype(BF)
    sh = {
        "Wg_s": np.ascontiguousarray(np.asarray(inputs["Wg_s"], np.float32).astype(BF)),
        "Wu_s": np.ascontiguousarray(np.asarray(inputs["Wu_s"], np.float32).astype(BF)),
        "Wd_s": np.ascontiguousarray(np.asarray(inputs["Wd_s"], np.float32).astype(BF)),
    }

    in_maps = []
    for c in range(NCORES):
        g = c // 2
        rows = core_rows[c]
        xrT = np.zeros((H, R), BF)
        xrT[:, :len(rows)] = xb[rows].T
        cw2 = np.zeros((R, 2), np.float32)
        cw2[:len(rows), 0] = cw_full[rows, 2 * g]
        cw2[:len(rows), 1] = cw_full[rows, 2 * g + 1]
        m = dict(sh)
        m["xrT"] = xrT
        m["xsT"] = np.ascontiguousarray(xb[c * NTOK:(c + 1) * NTOK].T)
        m["cw"] = cw2
        m["Wg2"] = np.ascontiguousarray(Wg[[2 * g, 2 * g + 1]])
        m["Wu2"] = np.ascontiguousarray(Wu[[2 * g, 2 * g + 1]])
        m["Wd2"] = np.ascontiguousarray(Wd[[2 * g, 2 * g + 1]])
        in_maps.append(m)

    nc = _get_nc(rt)
    res = run_bass_kernel_spmd(nc, in_maps, core_ids=list(range(NCORES)), trace=TRACE)
    LAST_RESULT = res

    out = np.zeros((N, H), np.float32)
    for c in range(NCORES):
        out[c * NTOK:(c + 1) * NTOK] += res.results[c]["out_s"]
        rows = core_rows[c]
        out[rows] += res.results[c]["out_r"][:len(rows)]
    return out.reshape(B, T, H).astype(np.float32)


# revision 9
# speedup vs baseline: 1.1305x; 1.1305x over previous
"""MoE routing kernel for Trainium2 (Bass/Tile), 8 NeuronCores.

DeepSeek-style MoE block: sigmoid router with group-limited top-k (4 groups
of 2 experts, top-2 groups -> all 4 of their experts), 8 routed SwiGLU
experts (H=1024, I=512) with combine weights, plus a shared expert,
N=8192 tokens.

Strategy (v3, "pure-GEMM device"):
  - Group-expert-parallel: each of the 4 router groups is owned by 2 cores;
    the host replicates the reference's fp32 routing, assigns each token's
    rows to its two selected groups' cores (even/odd split), and ALSO
    computes the exact combine weights (sigmoid-score normalization) on the
    host. This is all part of the all-to-all token dispatch that the
    sharding hint sanctions host-side; none of it is device work.
  - The host additionally pre-transposes the token activations, so the
    device kernel is nothing but expert GEMM streaming: no PE transposes,
    no on-chip router, no top-k compare chains. Per core: 2 routed experts
    over RT*128 rows (RT sized exactly from the realized routing, ~2050
    rows) + the shared expert over a dense 1024-token shard.
  - All expert matmuls run in bf16 (weights and activations host-rounded;
    ~1e-3 relative error vs the fp32 reference, well under the 2e-2 gate).
    bf16 keeps the PE at 1 row/cycle even for narrow tails, enables fast
    weight load (FWL), and halves DMA traffic vs fp32.
  - PSUM f32 accumulation throughout; combine weights applied during the
    down-projection drain with per-partition-scalar DVE ops; partial
    outputs summed on the host.
  - x rides the ACT HWDGE DMA ring, weights and stores the SP ring.
"""

import math

import numpy as np
import ml_dtypes

import concourse.bass as bass
import concourse.bacc as bacc
import concourse.tile as tile
from concourse import mybir
from concourse.bass_utils import run_bass_kernel_spmd

F32 = mybir.dt.float32
BF16 = mybir.dt.bfloat16
AF = mybir.ActivationFunctionType
ALU = mybir.AluOpType

B, T, H, I, E = 32, 256, 1024, 512, 8
N = B * T                     # 8192 tokens
NCORES = 8
NTOK = N // NCORES            # 1024 dense tokens per core (shared expert)
HK = H // 128                 # 8 contraction chunks over H
IK = I // 128                 # 4 chunks over I
SCALE = 2.5
BF = ml_dtypes.bfloat16

TRACE = False
LAST_RESULT = None
_NC_CACHE = {}


def _blocks(ntiles):
    """Split ntiles 128-row tiles into blocks of <=4 tiles (<=512 rows)."""
    out = [4] * (ntiles // 4)
    if ntiles % 4:
        out.append(ntiles % 4)
    return out


def _build_kernel(rt):
    """rt: number of 128-row tiles in the routed phase (per core)."""
    R = rt * 128
    nc = bacc.Bacc("TRN2", target_bir_lowering=False)

    xrT_d = nc.dram_tensor("xrT", [H, R], BF16, kind="ExternalInput")
    xsT_d = nc.dram_tensor("xsT", [H, NTOK], BF16, kind="ExternalInput")
    cw_d = nc.dram_tensor("cw", [R, 2], F32, kind="ExternalInput")
    wg_d = nc.dram_tensor("Wg2", [2, H, I], BF16, kind="ExternalInput")
    wu_d = nc.dram_tensor("Wu2", [2, H, I], BF16, kind="ExternalInput")
    wd_d = nc.dram_tensor("Wd2", [2, I, H], BF16, kind="ExternalInput")
    wgs_d = nc.dram_tensor("Wg_s", [H, I], BF16, kind="ExternalInput")
    wus_d = nc.dram_tensor("Wu_s", [H, I], BF16, kind="ExternalInput")
    wds_d = nc.dram_tensor("Wd_s", [I, H], BF16, kind="ExternalInput")
    outr_d = nc.dram_tensor("out_r", [R, H], F32, kind="ExternalOutput")
    outs_d = nc.dram_tensor("out_s", [NTOK, H], F32, kind="ExternalOutput")

    with tile.TileContext(nc) as tc:
        with (
            tc.tile_pool(name="wt", bufs=1) as p_wt,
            tc.tile_pool(name="cw", bufs=1) as p_cw,
            tc.tile_pool(name="xT", bufs=3) as p_xT,
            tc.tile_pool(name="sg", bufs=4) as p_sg,
            tc.tile_pool(name="h", bufs=2) as p_h,
            tc.tile_pool(name="acc", bufs=2) as p_acc,
            tc.tile_pool(name="st", bufs=6) as p_st,
            tc.tile_pool(name="psA", bufs=4, space="PSUM") as p_psA,
            tc.tile_pool(name="psY", bufs=2, space="PSUM") as p_psY,
        ):
            # combine weights: [R, 2] -> [128, rt, 2]
            cw_t = p_cw.tile([128, rt, 2], F32, tag="cw")
            nc.sync.dma_start(
                out=cw_t[:, :, :],
                in_=cw_d.ap().rearrange("(rt p) k -> p rt k", p=128),
            )
            cw_f = cw_t.rearrange("p rt k -> p (rt k)")

            # resident expert weights (bf16); one tag per tile (all live).
            # Spread across three HWDGE rings, ordered by first consumption,
            # so the PE never waits long for the next weight tile.
            def load_gu(dram, idx2, tag, eng):
                t = p_wt.tile([128, HK, I], BF16, tag=tag)
                src = dram.ap() if idx2 is None else dram.ap()[idx2]
                eng.dma_start(
                    out=t[:, :, :], in_=src.rearrange("(hk p) i -> p hk i", p=128)
                )
                return t

            def load_wd(dram, idx2, tag, eng):
                t = p_wt.tile([128, IK, H], BF16, tag=tag)
                src = dram.ap() if idx2 is None else dram.ap()[idx2]
                eng.dma_start(
                    out=t[:, :, :], in_=src.rearrange("(kc p) h -> p kc h", p=128)
                )
                return t

            wu2 = [load_gu(wu_d, 0, "wu0", nc.sync), None]
            wg2 = [load_gu(wg_d, 0, "wg0", nc.sync), None]
            wd2 = [load_wd(wd_d, 0, "wd0", nc.gpsimd), None]
            wu2[1] = load_gu(wu_d, 1, "wu1", nc.gpsimd)
            wg2[1] = load_gu(wg_d, 1, "wg1", nc.gpsimd)
            wd2[1] = load_wd(wd_d, 1, "wd1", nc.gpsimd)
            wus = load_gu(wus_d, None, "wus", nc.sync)
            wgs = load_gu(wgs_d, None, "wgs", nc.sync)
            wds = load_wd(wds_d, None, "wds", nc.gpsimd)

            def expert_block(xT, ntile, wg, wu, wd, combine):
                """SwiGLU for one expert over one <=512-token block.
                combine(m, y_psum) drains each 128-token down-proj result."""
                TBb = ntile * 128
                h = p_h.tile([128, IK, TBb], BF16, tag="h")
                for ik in range(IK):
                    ps_u = p_psA.tile([128, TBb], F32, tag="gu")
                    for hk in range(HK):
                        nc.tensor.matmul(
                            ps_u[:, :], wu[:, hk, ik * 128:(ik + 1) * 128],
                            xT[:, hk, 0:TBb], start=(hk == 0), stop=(hk == HK - 1),
                        )
                    ps_g = p_psA.tile([128, TBb], F32, tag="gu")
                    for hk in range(HK):
                        nc.tensor.matmul(
                            ps_g[:, :], wg[:, hk, ik * 128:(ik + 1) * 128],
                            xT[:, hk, 0:TBb], start=(hk == 0), stop=(hk == HK - 1),
                        )
                    sg = p_sg.tile([128, TBb], F32, tag="sg")
                    nc.scalar.activation(sg[:, :], ps_g[:, :], AF.Silu)
                    nc.vector.tensor_tensor(
                        h[:, ik, :], sg[:, :], ps_u[:, :], ALU.mult
                    )
                for m in range(ntile):
                    y = p_psY.tile([128, H], F32, tag="y")
                    for ik in range(IK):
                        lhsT = h[:, ik, m * 128:(m + 1) * 128]
                        for nh in range(2):
                            nc.tensor.matmul(
                                y[:, nh * 512:(nh + 1) * 512],
                                lhsT,
                                wd[:, ik, nh * 512:(nh + 1) * 512],
                                start=(ik == 0),
                                stop=(ik == IK - 1),
                            )
                    combine(m, y)

            # ---------------- phase 1: routed rows ----------------
            tt0 = 0
            for ntile in _blocks(rt):
                t0 = tt0 * 128
                TBb = ntile * 128
                xT = p_xT.tile([128, HK, TBb], BF16, tag="xT")
                nc.scalar.dma_start(
                    out=xT[:, :, :],
                    in_=xrT_d.ap().rearrange("(hk p) t -> p hk t", p=128)[
                        :, :, t0:t0 + TBb
                    ],
                )
                acc = p_acc.tile([128, ntile, H], F32, tag="acc")
                for slot in (0, 1):
                    def combine(m, y, slot=slot, acc=acc, tt0=tt0):
                        a = acc[:, m, :].squeeze()
                        col = cw_f[:, (tt0 + m) * 2 + slot:(tt0 + m) * 2 + slot + 1]
                        if slot == 0:
                            nc.vector.tensor_scalar(
                                a, y[:, :], col, None, ALU.mult
                            )
                        else:
                            nc.vector.scalar_tensor_tensor(
                                a, y[:, :], col, a, ALU.mult, ALU.add
                            )
                    expert_block(xT, ntile, wg2[slot], wu2[slot], wd2[slot], combine)
                for m in range(ntile):
                    tt = tt0 + m
                    nc.sync.dma_start(
                        out=outr_d.ap()[tt * 128:(tt + 1) * 128, :],
                        in_=acc[:, m, :].squeeze(),
                    )
                tt0 += ntile

            # ---------------- phase 2: shared expert ----------------
            st0 = 0
            for ntile in _blocks(NTOK // 128):
                t0 = st0 * 128
                TBb = ntile * 128
                xT = p_xT.tile([128, HK, TBb], BF16, tag="xT")
                nc.scalar.dma_start(
                    out=xT[:, :, :],
                    in_=xsT_d.ap().rearrange("(hk p) t -> p hk t", p=128)[
                        :, :, t0:t0 + TBb
                    ],
                )
                def combine(m, y, st0=st0):
                    tt = st0 + m
                    stage = p_st.tile([128, H], F32, tag="st")
                    nc.scalar.activation(stage[:, :], y[:, :], AF.Copy)
                    nc.sync.dma_start(
                        out=outs_d.ap()[tt * 128:(tt + 1) * 128, :],
                        in_=stage[:, :],
                    )
                expert_block(xT, ntile, wgs, wus, wds, combine)
                st0 += ntile

    if not nc.is_finalized():
        nc.finalize()
    return nc


def _get_nc(rt):
    if rt not in _NC_CACHE:
        _NC_CACHE[rt] = _build_kernel(rt)
    return _NC_CACHE[rt]


def kernel(**inputs):
    global LAST_RESULT
    hs = np.asarray(inputs["hidden_states"], np.float32)
    x = np.ascontiguousarray(hs.reshape(N, H))
    gw = np.ascontiguousarray(np.asarray(inputs["gate_w"], np.float32))
    cb = np.ascontiguousarray(np.asarray(inputs["correction_bias"], np.float32))

    # ---- host router: replicate the reference's fp32 group top-2 choice ----
    logits = x @ gw.T                                            # [N, E] f32
    scores = (1.0 / (1.0 + np.exp(-logits.astype(np.float64)))).astype(np.float32)
    sc = scores + cb
    gs = sc.reshape(N, 4, 2).sum(-1, dtype=np.float32)           # [N, 4]
    order = np.argsort(-gs, axis=1, kind="stable")
    sel = np.zeros((N, 4), bool)
    sel[np.arange(N)[:, None], order[:, :2]] = True              # [N, 4] groups
    sel_e = np.repeat(sel, 2, axis=1)                            # [N, E]
    w4 = np.where(sel_e, scores, 0.0).astype(np.float32)
    denom = w4.sum(1, dtype=np.float32) + np.float32(1e-20)
    cw_full = (w4 / denom[:, None] * np.float32(SCALE)).astype(np.float32)

    # ---- shard: rows of group g split even/odd between cores 2g, 2g+1 ----
    core_rows = []
    for c in range(NCORES):
        g, hlf = c // 2, c % 2
        core_rows.append(np.flatnonzero(sel[:, g])[hlf::2])
    rt = max(1, max(int(math.ceil(len(r) / 128)) for r in core_rows))
    R = rt * 128

    xb = x.astype(BF)
    Wg = np.asarray(inputs["Wg"], np.float32).astype(BF)
    Wu = np.asarray(inputs["Wu"], np.float32).astype(BF)
    Wd = np.asarray(inputs["Wd"], np.float32).astype(BF)
    sh = {
        "Wg_s": np.ascontiguousarray(np.asarray(inputs["Wg_s"], np.float32).astype(BF)),
        "Wu_s": np.ascontiguousarray(np.asarray(inputs["Wu_s"], np.float32).astype(BF)),
        "Wd_s": np.ascontiguousarray(np.asarray(inputs["Wd_s"], np.float32).astype(BF)),
    }

    in_maps = []
    for c in range(NCORES):
        g = c // 2
        rows = core_rows[c]
        xrT = np.zeros((H, R), BF)
        xrT[:, :len(rows)] = xb[rows].T
        cw2 = np.zeros((R, 2), np.float32)
        cw2[:len(rows), 0] = cw_full[rows, 2 * g]
        cw2[:len(rows), 1] = cw_full[rows, 2 * g + 1]
        m = dict(sh)
        m["xrT"] = xrT
        m["xsT"] = np.ascontiguousarray(xb[c * NTOK:(c + 1) * NTOK].T)
        m["cw"] = cw2
        m["Wg2"] = np.ascontiguousarray(Wg[[2 * g, 2 * g + 1]])
        m["Wu2"] = np.ascontiguousarray(Wu[[2 * g, 2 * g + 1]])
        m["Wd2"] = np.ascontiguousarray(Wd[[2 * g, 2 * g + 1]])
        in_maps.append(m)

    nc = _get_nc(rt)
    res = run_bass_kernel_spmd(nc, in_maps, core_ids=list(range(NCORES)), trace=TRACE)
    LAST_RESULT = res

    out = np.zeros((N, H), np.float32)
    for c in range(NCORES):
        out[c * NTOK:(c + 1) * NTOK] += res.results[c]["out_s"]
        rows = core_rows[c]
        out[rows] += res.results[c]["out_r"][:len(rows)]
    return out.reshape(B, T, H).astype(np.float32)


# revision 13
# speedup vs baseline: 1.1986x; 1.0603x over previous
"""MoE routing kernel for Trainium2 (Bass/Tile), 8 NeuronCores.

DeepSeek-style MoE block: sigmoid router with group-limited top-k (4 groups
of 2 experts, top-2 groups -> all 4 of their experts), 8 routed SwiGLU
experts (H=1024, I=512) with combine weights, plus a shared expert,
N=8192 tokens.

Strategy (v3, "pure-GEMM device"):
  - Group-expert-parallel: each of the 4 router groups is owned by 2 cores;
    the host replicates the reference's fp32 routing, assigns each token's
    rows to its two selected groups' cores (even/odd split), and ALSO
    computes the exact combine weights (sigmoid-score normalization) on the
    host. This is all part of the all-to-all token dispatch that the
    sharding hint sanctions host-side; none of it is device work.
  - The host additionally pre-transposes the token activations, so the
    device kernel is nothing but expert GEMM streaming: no PE transposes,
    no on-chip router, no top-k compare chains. Per core: 2 routed experts
    over RT*128 rows (RT sized exactly from the realized routing, ~2050
    rows) + the shared expert over a dense 1024-token shard.
  - All expert matmuls run in bf16 (weights and activations host-rounded;
    ~1e-3 relative error vs the fp32 reference, well under the 2e-2 gate).
    bf16 keeps the PE at 1 row/cycle even for narrow tails, enables fast
    weight load (FWL), and halves DMA traffic vs fp32.
  - PSUM f32 accumulation throughout; combine weights applied during the
    down-projection drain with per-partition-scalar DVE ops; partial
    outputs summed on the host.
  - x rides the ACT HWDGE DMA ring, weights and stores the SP ring.
"""

import math

import numpy as np
import ml_dtypes

import concourse.bass as bass
import concourse.bacc as bacc
import concourse.tile as tile
from concourse import mybir
from concourse.bass_utils import run_bass_kernel_spmd

F32 = mybir.dt.float32
BF16 = mybir.dt.bfloat16
AF = mybir.ActivationFunctionType
ALU = mybir.AluOpType

B, T, H, I, E = 32, 256, 1024, 512, 8
N = B * T                     # 8192 tokens
NCORES = 8
NTOK = N // NCORES            # 1024 dense tokens per core (shared expert)
HK = H // 128                 # 8 contraction chunks over H
IK = I // 128                 # 4 chunks over I
SCALE = 2.5
BF = ml_dtypes.bfloat16

TRACE = False
LAST_RESULT = None
_NC_CACHE = {}


def _blocks(ntiles):
    """Split ntiles 128-row tiles into blocks of <=4 tiles (<=512 rows)."""
    out = [4] * (ntiles // 4)
    if ntiles % 4:
        out.append(ntiles % 4)
    return out


def _build_kernel(rt):
    """rt: number of 128-row tiles in the routed phase (per core)."""
    R = rt * 128
    nc = bacc.Bacc("TRN2", target_bir_lowering=False)

    xrT_d = nc.dram_tensor("xrT", [H, R], BF16, kind="ExternalInput")
    xsT_d = nc.dram_tensor("xsT", [H, NTOK], BF16, kind="ExternalInput")
    cw_d = nc.dram_tensor("cw", [R, 2], F32, kind="ExternalInput")
    wg_d = nc.dram_tensor("Wg2", [2, H, I], BF16, kind="ExternalInput")
    wu_d = nc.dram_tensor("Wu2", [2, H, I], BF16, kind="ExternalInput")
    wd_d = nc.dram_tensor("Wd2", [2, I, H], BF16, kind="ExternalInput")
    wgs_d = nc.dram_tensor("Wg_s", [H, I], BF16, kind="ExternalInput")
    wus_d = nc.dram_tensor("Wu_s", [H, I], BF16, kind="ExternalInput")
    wds_d = nc.dram_tensor("Wd_s", [I, H], BF16, kind="ExternalInput")
    outr_d = nc.dram_tensor("out_r", [R, H], F32, kind="ExternalOutput")
    outs_d = nc.dram_tensor("out_s", [NTOK, H], F32, kind="ExternalOutput")

    with tile.TileContext(nc) as tc:
        with (
            tc.tile_pool(name="wt", bufs=1) as p_wt,
            tc.tile_pool(name="cw", bufs=1) as p_cw,
            tc.tile_pool(name="xT", bufs=3) as p_xT,
            tc.tile_pool(name="sg", bufs=4) as p_sg,
            tc.tile_pool(name="h", bufs=2) as p_h,
            tc.tile_pool(name="acc", bufs=2) as p_acc,
            tc.tile_pool(name="st", bufs=6) as p_st,
            tc.tile_pool(name="psA", bufs=4, space="PSUM") as p_psA,
            tc.tile_pool(name="psY", bufs=2, space="PSUM") as p_psY,
        ):
            # combine weights: [R, 2] -> [128, rt, 2]
            cw_t = p_cw.tile([128, rt, 2], F32, tag="cw")
            nc.sync.dma_start(
                out=cw_t[:, :, :],
                in_=cw_d.ap().rearrange("(rt p) k -> p rt k", p=128),
            )
            cw_f = cw_t.rearrange("p rt k -> p (rt k)")

            # resident expert weights (bf16); one tag per tile (all live).
            # Spread across three HWDGE rings, ordered by first consumption,
            # so the PE never waits long for the next weight tile.
            def load_gu(dram, idx2, tag, eng):
                t = p_wt.tile([128, HK, I], BF16, tag=tag)
                src = dram.ap() if idx2 is None else dram.ap()[idx2]
                eng.dma_start(
                    out=t[:, :, :], in_=src.rearrange("(hk p) i -> p hk i", p=128)
                )
                return t

            def load_wd(dram, idx2, tag, eng):
                t = p_wt.tile([128, IK, H], BF16, tag=tag)
                src = dram.ap() if idx2 is None else dram.ap()[idx2]
                eng.dma_start(
                    out=t[:, :, :], in_=src.rearrange("(kc p) h -> p kc h", p=128)
                )
                return t

            # all on the sync ring, in PE consumption order (both slots'
            # gate/up run before either slot's down-projection)
            wu2 = [load_gu(wu_d, 0, "wu0", nc.sync), None]
            wg2 = [load_gu(wg_d, 0, "wg0", nc.sync), None]
            wu2[1] = load_gu(wu_d, 1, "wu1", nc.sync)
            wg2[1] = load_gu(wg_d, 1, "wg1", nc.sync)
            wd2 = [load_wd(wd_d, 0, "wd0", nc.sync), load_wd(wd_d, 1, "wd1", nc.sync)]
            wus = load_gu(wus_d, None, "wus", nc.sync)
            wgs = load_gu(wgs_d, None, "wgs", nc.sync)
            wds = load_wd(wds_d, None, "wds", nc.sync)

            def gu_phase(xT, ntile, wg, wu):
                """gate/up + SwiGLU for one expert over one <=512-token
                block; returns the bf16 h tile."""
                TBb = ntile * 128
                h = p_h.tile([128, IK, TBb], BF16, tag="h")
                for ik in range(IK):
                    ps_u = p_psA.tile([128, TBb], F32, tag="gu")
                    for hk in range(HK):
                        nc.tensor.matmul(
                            ps_u[:, :], wu[:, hk, ik * 128:(ik + 1) * 128],
                            xT[:, hk, 0:TBb], start=(hk == 0), stop=(hk == HK - 1),
                        )
                    ps_g = p_psA.tile([128, TBb], F32, tag="gu")
                    for hk in range(HK):
                        nc.tensor.matmul(
                            ps_g[:, :], wg[:, hk, ik * 128:(ik + 1) * 128],
                            xT[:, hk, 0:TBb], start=(hk == 0), stop=(hk == HK - 1),
                        )
                    sg = p_sg.tile([128, TBb], F32, tag="sg")
                    nc.scalar.activation(sg[:, :], ps_g[:, :], AF.Silu)
                    nc.vector.tensor_tensor(
                        h[:, ik, :], sg[:, :], ps_u[:, :], ALU.mult
                    )
                return h

            def down_phase(h, ntile, wd, combine):
                """down-projection; combine(m, y_psum) drains each
                128-token result tile."""
                for m in range(ntile):
                    y = p_psY.tile([128, H], F32, tag="y")
                    for ik in range(IK):
                        lhsT = h[:, ik, m * 128:(m + 1) * 128]
                        for nh in range(2):
                            nc.tensor.matmul(
                                y[:, nh * 512:(nh + 1) * 512],
                                lhsT,
                                wd[:, ik, nh * 512:(nh + 1) * 512],
                                start=(ik == 0),
                                stop=(ik == IK - 1),
                            )
                    combine(m, y)

            # ---------------- phase 1: routed rows ----------------
            tt0 = 0
            for ntile in _blocks(rt):
                t0 = tt0 * 128
                TBb = ntile * 128
                xT = p_xT.tile([128, HK, TBb], BF16, tag="xT")
                nc.gpsimd.dma_start(
                    out=xT[:, :, :],
                    in_=xrT_d.ap().rearrange("(hk p) t -> p hk t", p=128)[
                        :, :, t0:t0 + TBb
                    ],
                )
                acc = p_acc.tile([128, ntile, H], F32, tag="acc")

                def mk_combine(slot, acc=acc, tt0=tt0):
                    def combine(m, y):
                        a = acc[:, m, :].squeeze()
                        col = cw_f[:, (tt0 + m) * 2 + slot:(tt0 + m) * 2 + slot + 1]
                        if slot == 0:
                            nc.vector.tensor_scalar(
                                a, y[:, :], col, None, ALU.mult
                            )
                        else:
                            nc.vector.scalar_tensor_tensor(
                                a, y[:, :], col, a, ALU.mult, ALU.add
                            )
                    return combine

                h0 = gu_phase(xT, ntile, wg2[0], wu2[0])
                h1 = gu_phase(xT, ntile, wg2[1], wu2[1])
                down_phase(h0, ntile, wd2[0], mk_combine(0))
                down_phase(h1, ntile, wd2[1], mk_combine(1))
                for m in range(ntile):
                    tt = tt0 + m
                    nc.sync.dma_start(
                        out=outr_d.ap()[tt * 128:(tt + 1) * 128, :],
                        in_=acc[:, m, :].squeeze(),
                    )
                tt0 += ntile

            # ---------------- phase 2: shared expert ----------------
            st0 = 0
            for ntile in _blocks(NTOK // 128):
                t0 = st0 * 128
                TBb = ntile * 128
                xT = p_xT.tile([128, HK, TBb], BF16, tag="xT")
                nc.gpsimd.dma_start(
                    out=xT[:, :, :],
                    in_=xsT_d.ap().rearrange("(hk p) t -> p hk t", p=128)[
                        :, :, t0:t0 + TBb
                    ],
                )
                def combine(m, y, st0=st0):
                    tt = st0 + m
                    stage = p_st.tile([128, H], F32, tag="st")
                    nc.scalar.activation(stage[:, :], y[:, :], AF.Copy)
                    nc.sync.dma_start(
                        out=outs_d.ap()[tt * 128:(tt + 1) * 128, :],
                        in_=stage[:, :],
                    )
                h = gu_phase(xT, ntile, wgs, wus)
                down_phase(h, ntile, wds, combine)
                st0 += ntile

    if not nc.is_finalized():
        nc.finalize()
    return nc


def _get_nc(rt):
    if rt not in _NC_CACHE:
        _NC_CACHE[rt] = _build_kernel(rt)
    return _NC_CACHE[rt]


def kernel(**inputs):
    global LAST_RESULT
    hs = np.asarray(inputs["hidden_states"], np.float32)
    x = np.ascontiguousarray(hs.reshape(N, H))
    gw = np.ascontiguousarray(np.asarray(inputs["gate_w"], np.float32))
    cb = np.ascontiguousarray(np.asarray(inputs["correction_bias"], np.float32))

    # ---- host router: replicate the reference's fp32 group top-2 choice ----
    logits = x @ gw.T                                            # [N, E] f32
    scores = (1.0 / (1.0 + np.exp(-logits.astype(np.float64)))).astype(np.float32)
    sc = scores + cb
    gs = sc.reshape(N, 4, 2).sum(-1, dtype=np.float32)           # [N, 4]
    order = np.argsort(-gs, axis=1, kind="stable")
    sel = np.zeros((N, 4), bool)
    sel[np.arange(N)[:, None], order[:, :2]] = True              # [N, 4] groups
    sel_e = np.repeat(sel, 2, axis=1)                            # [N, E]
    w4 = np.where(sel_e, scores, 0.0).astype(np.float32)
    denom = w4.sum(1, dtype=np.float32) + np.float32(1e-20)
    cw_full = (w4 / denom[:, None] * np.float32(SCALE)).astype(np.float32)

    # ---- shard: rows of group g split even/odd between cores 2g, 2g+1 ----
    core_rows = []
    for c in range(NCORES):
        g, hlf = c // 2, c % 2
        core_rows.append(np.flatnonzero(sel[:, g])[hlf::2])
    rt = max(1, max(int(math.ceil(len(r) / 128)) for r in core_rows))
    R = rt * 128

    xb = x.astype(BF)
    Wg = np.asarray(inputs["Wg"], np.float32).astype(BF)
    Wu = np.asarray(inputs["Wu"], np.float32).astype(BF)
    Wd = np.asarray(inputs["Wd"], np.float32).astype(BF)
    sh = {
        "Wg_s": np.ascontiguousarray(np.asarray(inputs["Wg_s"], np.float32).astype(BF)),
        "Wu_s": np.ascontiguousarray(np.asarray(inputs["Wu_s"], np.float32).astype(BF)),
        "Wd_s": np.ascontiguousarray(np.asarray(inputs["Wd_s"], np.float32).astype(BF)),
    }

    in_maps = []
    for c in range(NCORES):
        g = c // 2
        rows = core_rows[c]
        xrT = np.zeros((H, R), BF)
        xrT[:, :len(rows)] = xb[rows].T
        cw2 = np.zeros((R, 2), np.float32)
        cw2[:len(rows), 0] = cw_full[rows, 2 * g]
        cw2[:len(rows), 1] = cw_full[rows, 2 * g + 1]
        m = dict(sh)
        m["xrT"] = xrT
        m["xsT"] = np.ascontiguousarray(xb[c * NTOK:(c + 1) * NTOK].T)
        m["cw"] = cw2
        m["Wg2"] = np.ascontiguousarray(Wg[[2 * g, 2 * g + 1]])
        m["Wu2"] = np.ascontiguousarray(Wu[[2 * g, 2 * g + 1]])
        m["Wd2"] = np.ascontiguousarray(Wd[[2 * g, 2 * g + 1]])
        in_maps.append(m)

    nc = _get_nc(rt)
    res = run_bass_kernel_spmd(nc, in_maps, core_ids=list(range(NCORES)), trace=TRACE)
    LAST_RESULT = res

    out = np.zeros((N, H), np.float32)
    for c in range(NCORES):
        out[c * NTOK:(c + 1) * NTOK] += res.results[c]["out_s"]
        rows = core_rows[c]
        out[rows] += res.results[c]["out_r"][:len(rows)]
    return out.reshape(B, T, H).astype(np.float32)
